# revision 9
# baseline (speedup 1.0000x reference)
"""Trainium2 Bass kernel for nn_GRUForecaster (2-layer GRU + FC head).

Sharding: data-parallel across 8 NeuronCores (batch 32 -> 4 rows/core),
weights replicated. Each core runs: xi0 GEMM -> L0 recurrence -> xi1 GEMM
-> L1 recurrence -> FC on last hidden state.

Self-contained: hardcodes shapes from the problem spec.
"""
import sys

sys.path.insert(0, "/opt/trn_rl_repo")

import numpy as np

import concourse.bass as bass
import concourse.mybir as mybir
import concourse.tile as tile
from concourse.bass import ds
from concourse.masks import make_identity

F32 = mybir.dt.float32
AF = mybir.ActivationFunctionType
ALU = mybir.AluOpType

B, T, I, H, O = 32, 2048, 256, 512, 64
H3 = 3 * H
NCORES = 8
BL = B // NCORES  # batch rows per core
UNROLL = 4  # time steps per For_i iteration
KC = H // 128  # 4 k-chunks of the hidden contraction
KCI = I // 128  # 2 k-chunks of the input contraction


def _split_multiwaits(nc, max_waits=1):
    """This container's walrus build supports only one sync-wait command per
    instruction; Tile's epilogue drain can carry several. Hoist excess waits
    onto dedicated single-wait CTRL instructions just before the offender."""
    counter = [0]

    def fresh_name():
        counter[0] += 1
        return f"I-waitsplit-{counter[0]}"

    for f in nc.m.functions:
        for blk in f.blocks:
            new_insts = []
            for inst in blk.instructions:
                si = inst.sync_info
                if si is not None and si.on_wait is not None and len(si.on_wait) > max_waits:
                    waits = list(si.on_wait)
                    for w in waits[:-max_waits]:
                        new_insts.append(
                            mybir.InstEventSemaphore(
                                name=fresh_name(),
                                opcode="EventSemaphore",
                                engine=inst.engine,
                                ins=[],
                                outs=[],
                                sync_info=mybir.SyncInfo(on_wait=[w], on_update=[]),
                            )
                        )
                    si.on_wait = waits[-max_waits:]
                new_insts.append(inst)
            blk.instructions = new_insts


def _input_gemm(nc, tc, pools, psum, x_dram, w_sb, bias_sb, xi_dram, t_steps, kc_in, pre_t):
    """xi = x @ W + bias, written to xi_dram [BL, t_steps, 3H].

    x_dram: [BL, t_steps, kc_in*128] natural layout (needs PE transpose), or
    pre-transposed DRAM [128, KC, BL, t_steps] when pre_t is True.
    w_sb: [128, kc_in, H3] sbuf weights. bias_sb: [1, H3].
    """
    sing, work = pools
    i128 = sing["i128"]
    ones128 = sing["ones128"]
    for b in range(BL):
        for tc_i in range(t_steps // 128):
            t0 = tc_i * 128
            xT = work.tile([128, kc_in, 128], F32, tag="gemm_xT", bufs=2)
            if pre_t:
                nc.sync.dma_start(
                    out=xT, in_=x_dram[:, :, b, t0 : t0 + 128]
                )
            else:
                pst = psum.tile([128, kc_in, 128], F32, tag="gemm_pst")
                for kc in range(kc_in):
                    xt = work.tile([128, 128], F32, tag="gemm_xt")
                    nc.sync.dma_start(
                        out=xt, in_=x_dram[b, t0 : t0 + 128, kc * 128 : (kc + 1) * 128]
                    )
                    nc.tensor.transpose(pst[:, kc, :], xt, i128)
                nc.vector.tensor_copy(xT, pst)
            pxi = psum.tile([128, H3], F32, tag="gemm_pxi")
            for kc in range(kc_in):
                for nb in range(3):
                    nc.tensor.matmul(
                        pxi[:, nb * 512 : (nb + 1) * 512],
                        xT[:, kc, :],
                        w_sb[:, kc, nb * 512 : (nb + 1) * 512],
                        start=(kc == 0),
                        stop=False,
                    )
            for nb in range(3):
                nc.tensor.matmul(
                    pxi[:, nb * 512 : (nb + 1) * 512],
                    ones128,
                    bias_sb[:, nb * 512 : (nb + 1) * 512],
                    start=False,
                    stop=True,
                )
            xi_sb = work.tile([128, H3], F32, tag="gemm_xi", bufs=2)
            nc.scalar.copy(xi_sb[:, 0:768], pxi[:, 0:768])
            nc.vector.tensor_copy(xi_sb[:, 768:H3], pxi[:, 768:H3])
            xi_view = xi_dram.rearrange("(t b) n -> t b n", b=BL)
            nc.sync.dma_start(out=xi_view[t0 : t0 + 128, b, :], in_=xi_sb)


def _recurrence(nc, tc, pools, psum, xi_dram, wh_sb, bhn_sb, h_nat, hT_sb, h0T_dram):
    """One GRU layer recurrence over T steps. h_nat [BL,H], hT_sb [128, KC*BL]
    are persistent state tiles (already zeroed). If h0T_dram is not None the
    transposed hidden states are streamed out for the next layer's GEMM."""
    sing, work = pools
    i4 = sing["i4"]
    ones1 = sing["ones1"]

    xi_view = xi_dram.rearrange("(t b) n -> t b n", b=BL)
    with tc.For_i(0, T, UNROLL) as t0:
        ring = None
        if h0T_dram is not None:
            ring = work.tile([128, KC, BL, UNROLL], F32, tag="rec_ring")
        xi_c = work.tile([BL, UNROLL, H3], F32, tag="rec_xi", bufs=2)
        nc.sync.dma_start(
            out=xi_c, in_=xi_view[ds(t0, UNROLL), :, :].rearrange("t b n -> b t n")
        )
        for j in range(UNROLL):
            xi_t = xi_c[:, j, :]
            pu = psum.tile([BL, H3], F32, tag="rec_pu")
            for kc in range(KC):
                for nb in range(3):
                    nc.tensor.matmul(
                        pu[:, nb * 512 : (nb + 1) * 512],
                        hT_sb[:, kc, :],
                        wh_sb[:, kc, nb * 512 : (nb + 1) * 512],
                        start=(kc == 0),
                        stop=False,
                    )
            # accumulate xi into the r,z banks; bhn into the n bank
            for nb in range(2):
                nc.tensor.matmul(
                    pu[:, nb * 512 : (nb + 1) * 512],
                    i4,
                    xi_t[:, nb * 512 : (nb + 1) * 512],
                    start=False,
                    stop=True,
                )
            nc.tensor.matmul(
                pu[:, 1024:1536], ones1, bhn_sb, start=False, stop=True
            )
            rz = work.tile([BL, 1024], F32, tag="rec_rz", bufs=2)
            nc.scalar.activation(rz, pu[:, 0:1024], AF.Sigmoid)
            v = work.tile([BL, H], F32, tag="rec_v", bufs=2)
            nc.vector.tensor_mul(v, rz[:, 0:H], pu[:, 1024:1536])
            nin = work.tile([BL, H], F32, tag="rec_nin", bufs=2)
            nc.vector.tensor_add(nin, v, xi_t[:, 1024:1536])
            n_sb = work.tile([BL, H], F32, tag="rec_n", bufs=2)
            nc.scalar.activation(n_sb, nin, AF.Tanh)
            d = work.tile([BL, H], F32, tag="rec_d", bufs=2)
            nc.vector.scalar_tensor_tensor(
                d, n_sb, -1.0, h_nat, op0=ALU.mult, op1=ALU.add
            )
            e = work.tile([BL, H], F32, tag="rec_e", bufs=2)
            nc.vector.tensor_mul(e, rz[:, H : 2 * H], d)
            nc.vector.tensor_add(h_nat, n_sb, e)
            pT = psum.tile([128, KC, BL], F32, tag="rec_pT")
            for kc in range(KC):
                nc.tensor.transpose(
                    pT[:, kc, :], h_nat[:, kc * 128 : (kc + 1) * 128], i4
                )
            nc.vector.tensor_copy(hT_sb, pT)
            if ring is not None:
                nc.scalar.copy(ring[:, :, :, j], pT)
        if ring is not None:
            nc.sync.dma_start(out=h0T_dram[:, :, :, ds(t0, UNROLL)], in_=ring)


def build_nc():
    nc = bass.Bass()
    x = nc.dram_tensor("x", [BL, T, I], F32, kind="ExternalInput")
    Wi0 = nc.dram_tensor("Wi0", [I, H3], F32, kind="ExternalInput")
    bi0 = nc.dram_tensor("bi0", [H3], F32, kind="ExternalInput")
    Wh0 = nc.dram_tensor("Wh0", [H, H3], F32, kind="ExternalInput")
    bhn0 = nc.dram_tensor("bhn0", [H], F32, kind="ExternalInput")
    Wi1 = nc.dram_tensor("Wi1", [H, H3], F32, kind="ExternalInput")
    bi1 = nc.dram_tensor("bi1", [H3], F32, kind="ExternalInput")
    Wh1 = nc.dram_tensor("Wh1", [H, H3], F32, kind="ExternalInput")
    bhn1 = nc.dram_tensor("bhn1", [H], F32, kind="ExternalInput")
    Wfc = nc.dram_tensor("Wfc", [H, O], F32, kind="ExternalInput")
    bfc = nc.dram_tensor("bfc", [O], F32, kind="ExternalInput")
    out = nc.dram_tensor("out", [BL, O], F32, kind="ExternalOutput")

    xi0_d = nc.dram_tensor("xi0_scratch", [T * BL, H3], F32)
    xi1_d = nc.dram_tensor("xi1_scratch", [T * BL, H3], F32)
    h0T_d = nc.dram_tensor("h0T_scratch", [128, KC, BL, T], F32)

    def bcast1(src):
        return bass.AP(tensor=src.tensor, offset=src.offset, ap=[[0, 1], *src.ap])

    with tile.TileContext(nc) as tc:
        with (
            tc.tile_pool(name="sing", bufs=1) as singp,
            tc.tile_pool(name="work", bufs=3) as work,
        ):
            sing = {}
            sing["i128"] = singp.tile([128, 128], F32, name="ident128")
            make_identity(nc, sing["i128"])
            sing["i4"] = singp.tile([BL, BL], F32, name="ident4")
            make_identity(nc, sing["i4"])
            sing["ones128"] = singp.tile([1, 128], F32, name="ones128")
            nc.vector.memset(sing["ones128"], 1.0)
            sing["ones1"] = singp.tile([1, BL], F32, name="ones1")
            nc.vector.memset(sing["ones1"], 1.0)

            wi0_sb = singp.tile([128, KCI, H3], F32)
            nc.sync.dma_start(
                out=wi0_sb, in_=Wi0.ap().rearrange("(c p) n -> p c n", p=128)
            )
            wh0_sb = singp.tile([128, KC, H3], F32)
            nc.sync.dma_start(
                out=wh0_sb, in_=Wh0.ap().rearrange("(c p) n -> p c n", p=128)
            )
            wi1_sb = singp.tile([128, KC, H3], F32)
            nc.sync.dma_start(
                out=wi1_sb, in_=Wi1.ap().rearrange("(c p) n -> p c n", p=128)
            )
            wh1_sb = singp.tile([128, KC, H3], F32)
            nc.sync.dma_start(
                out=wh1_sb, in_=Wh1.ap().rearrange("(c p) n -> p c n", p=128)
            )
            wfc_sb = singp.tile([128, KC, O], F32)
            nc.sync.dma_start(
                out=wfc_sb, in_=Wfc.ap().rearrange("(c p) n -> p c n", p=128)
            )
            bi0_sb = singp.tile([1, H3], F32)
            nc.sync.dma_start(out=bi0_sb, in_=bcast1(bi0.ap()))
            bi1_sb = singp.tile([1, H3], F32)
            nc.sync.dma_start(out=bi1_sb, in_=bcast1(bi1.ap()))
            bhn0_sb = singp.tile([1, H], F32)
            nc.sync.dma_start(out=bhn0_sb, in_=bcast1(bhn0.ap()))
            bhn1_sb = singp.tile([1, H], F32)
            nc.sync.dma_start(out=bhn1_sb, in_=bcast1(bhn1.ap()))
            bfc_sb = singp.tile([1, O], F32)
            nc.sync.dma_start(out=bfc_sb, in_=bcast1(bfc.ap()))

            h_nat = singp.tile([BL, H], F32)
            hT_sb = singp.tile([128, KC, BL], F32)

            pools = (sing, work)

            # layer 0 input projection
            with tc.tile_pool(name="psg0", bufs=2, space="PSUM") as psg:
                _input_gemm(nc, tc, pools, psg, x.ap(), wi0_sb, bi0_sb, xi0_d.ap(), T, KCI, pre_t=False)

            # layer 0 recurrence
            nc.vector.memset(h_nat, 0.0)
            nc.vector.memset(hT_sb, 0.0)
            with tc.tile_pool(name="psr0", bufs=2, space="PSUM") as psr:
                _recurrence(nc, tc, pools, psr, xi0_d.ap(), wh0_sb, bhn0_sb, h_nat, hT_sb, h0T_d.ap())

            # layer 1 input projection from transposed h0
            with tc.tile_pool(name="psg1", bufs=2, space="PSUM") as psg:
                _input_gemm(nc, tc, pools, psg, h0T_d.ap(), wi1_sb, bi1_sb, xi1_d.ap(), T, KC, pre_t=True)

            # layer 1 recurrence
            nc.vector.memset(h_nat, 0.0)
            nc.vector.memset(hT_sb, 0.0)
            with tc.tile_pool(name="psr1", bufs=2, space="PSUM") as psr:
                _recurrence(nc, tc, pools, psr, xi1_d.ap(), wh1_sb, bhn1_sb, h_nat, hT_sb, None)

            # FC head on the final hidden state
            with tc.tile_pool(name="psfc", bufs=1, space="PSUM") as psfcp:
                pfc = psfcp.tile([BL, O], F32, tag="fc")
                for kc in range(KC):
                    nc.tensor.matmul(
                        pfc,
                        hT_sb[:, kc, :],
                        wfc_sb[:, kc, :],
                        start=(kc == 0),
                        stop=False,
                    )
                nc.tensor.matmul(pfc, sing["ones1"], bfc_sb, start=False, stop=True)
                out_sb = work.tile([BL, O], F32, tag="fc_out")
                nc.vector.tensor_copy(out_sb, pfc)
                nc.sync.dma_start(out=out.ap(), in_=out_sb)

    _split_multiwaits(nc)
    return nc


_NC_CACHE = {}


def kernel(**inputs) -> np.ndarray:
    from concourse.bass_utils import run_bass_kernel_spmd

    if "nc" not in _NC_CACHE:
        _NC_CACHE["nc"] = build_nc()
    nc = _NC_CACHE["nc"]

    def f32c(a):
        return np.ascontiguousarray(np.asarray(a, dtype=np.float32))

    x = f32c(inputs["x"])
    shared = {
        k: f32c(inputs[k])
        for k in ("Wi0", "bi0", "Wh0", "bhn0", "Wi1", "bi1", "Wh1", "bhn1", "Wfc", "bfc")
    }
    in_maps = [
        {"x": np.ascontiguousarray(x[c * BL : (c + 1) * BL]), **shared}
        for c in range(NCORES)
    ]
    res = run_bass_kernel_spmd(nc, in_maps, core_ids=list(range(NCORES)))
    global LAST_RESULT
    LAST_RESULT = res
    return np.concatenate([r["out"] for r in res.results], axis=0)


LAST_RESULT = None
